# revision 1
# baseline (speedup 1.0000x reference)
"""MoE layer (B=2,T=1024,D=2048,F=768,E=16,K=2) on 8 NeuronCores.

Expert-parallel: 16 experts sorted by routed-token count; the 8 heaviest go
in slot 0 (capacity C0), the 8 lightest in slot 1 (capacity C1 <= C0), one
of each per core. Host computes the router (softmax -> top-2 -> renormalize,
~0.3% of FLOPs), gathers each expert's tokens into fixed-capacity transposed
buffers, and the device kernel runs the sparse SwiGLU FFN in bf16 with f32
PSUM accumulation. The per-token routing weight is applied on the host
during the scatter-add, so no combine-weight tensor ships to the device.

The kernel is simultaneously at the HBM roofline (~21MB in / ~350GB/s) and
the PE roofline (~66us of matmul issue), so the schedule is built around
stream deadlines:
- tokens staged transposed (xgt [D, C] as [P, KD*C]) so gate/up produce
  hT [F, C] directly in the lhsT layout the down projection wants.
- the down projection is STAGED: three passes of 2 f-chunks each, partial
  yT accumulated in SBUF bf16. This defers the deadline of down-weight
  slab h to (down_start + h*T_down/3), which is what makes the total
  input stream (~60us of wire time) fit before the final pass.
- sync HW-DGE queue: gate(e0), down(e0), gate(e1), down(e1) weight slabs;
  scalar HW-DGE queue: tokens(e0), up(e0), tokens(e1), up(e1) (the scalar
  engine also runs silu, so its later triggers are interleaved between
  evictions); gpsimd SWDGE: y(e0) out. y(e1) alternates sync/scalar in
  small batches for a short drain tail.
- both experts' full weight sets are double-buffered in SBUF (~188KB/part).
- ~8 garbage matmuls at kernel start warm the PE clock (HAM) during the
  initial DMA ramp.
"""

import numpy as np
from contextlib import ExitStack

import concourse.bass as bass
import concourse.tile as tile
from concourse import mybir
from concourse.bass_utils import run_bass_kernel_spmd

B, T, D, F, E, TOPK = 2, 1024, 2048, 768, 16, 2
NCORES = 8
EPC = E // NCORES  # experts per core (2 slots)
P = 128


def _split_waits(nc, max_waits=1):
    """walrus on this image rejects >1 sync-wait per instruction
    (setupSyncWait: "Too many sync wait commands"); split extras into
    preceding same-engine NoOps."""
    for f in nc.m.functions:
        for b in f.blocks:
            insts = b.instructions
            idx = 0
            while idx < len(insts):
                inst = insts[idx]
                si = getattr(inst, "sync_info", None)
                if si is not None and si.on_wait and len(si.on_wait) > max_waits:
                    waits = list(si.on_wait)
                    extra, keep = waits[:-max_waits], waits[-max_waits:]
                    pos = idx
                    for j in range(0, len(extra), max_waits):
                        chunk = extra[j : j + max_waits]
                        nop = mybir.InstNoOp(name=f"{inst.name}_ws{j}", ins=[], outs=[])
                        nop.engine = inst.engine
                        nop.sync_info = mybir.SyncInfo(on_wait=chunk, on_update=[])
                        insts.insert(pos, nop)
                        pos += 1
                        idx += 1
                    inst.sync_info = mybir.SyncInfo(
                        on_wait=keep, on_update=list(si.on_update)
                    )
                idx += 1


def build_moe(C0, C1):
    """Per-core kernel: slot 0 capacity C0, slot 1 capacity C1 (each %8==0)."""
    assert C0 % 8 == 0 and C1 % 8 == 0 and 128 <= C1 <= C0 <= 512
    CS = (C0, C1)
    KD = D // P  # 16 k-tiles over D
    KF = F // P  # 6 f-chunks over F
    MD = D // P  # 16 m-chunks over D (down proj, yT layout)
    XS = 4  # token DMA slabs per expert
    bf16 = mybir.dt.bfloat16
    f32 = mybir.dt.float32

    nc = bass.Bass("TRN2", target_bir_lowering=False, debug=False, num_devices=NCORES)
    # host pre-tiled layouts (>=2KB contiguous per partition per DMA):
    #   xgt{s}[p, k*C + c] = x_gathered[s, k*128+p, c]
    #   wg/wu[e, j, p, k*128+f] = w[e, k*128+p, j*128+f]     (slab per f-chunk j)
    xgt0 = nc.declare_dram_parameter("xgt0", [P, KD * C0], bf16, isOutput=False)
    xgt1 = nc.declare_dram_parameter("xgt1", [P, KD * C1], bf16, isOutput=False)
    wg = nc.declare_dram_parameter("wg", [EPC, KF, P, KD * P], bf16, isOutput=False)
    wu = nc.declare_dram_parameter("wu", [EPC, KF, P, KD * P], bf16, isOutput=False)
    wd = nc.declare_dram_parameter("wd", [EPC, F, D], bf16, isOutput=False)
    y0 = nc.declare_dram_parameter("y0", [D, C0], bf16, isOutput=True)
    y1 = nc.declare_dram_parameter("y1", [D, C1], bf16, isOutput=True)
    xgts = (xgt0, xgt1)
    ys = (y0, y1)

    with tile.TileContext(nc) as tc, ExitStack() as ctx:
        xp = ctx.enter_context(tc.tile_pool(name="xp", bufs=1))
        wgp = ctx.enter_context(tc.tile_pool(name="wgp", bufs=2))
        # dt tiles single-buffered: e1's down-weight DMA then naturally waits
        # for e0's pass-h reads to finish -- a free throttle that keeps that
        # traffic out of the oversubscribed pre-transition window.
        wdp = ctx.enter_context(tc.tile_pool(name="wdp", bufs=1))
        hp = ctx.enter_context(tc.tile_pool(name="hp", bufs=1))
        sp = ctx.enter_context(tc.tile_pool(name="sp", bufs=2))
        op = ctx.enter_context(tc.tile_pool(name="op", bufs=1))
        # one shared 8-bank PSUM ring: gate phase runs 4 j-iterations deep,
        # down phases run 8 accumulation groups ahead of the evictions.
        pp = ctx.enter_context(tc.tile_pool(name="pp", bufs=8, space="PSUM"))

        # PE warmup: garbage matmuls with no dependencies run during the
        # initial DMA ramp so HAM un-throttles (1.2->2.4GHz) before real work.
        wsb = sp.tile([P, 512], bf16, tag="warm_sb", bufs=1)
        nc.vector.memset(wsb[:], 0)
        # ~4us of garbage matmuls bridge the token/weight arrival ramp so
        # the first real matmuls run warm.
        for _ in range(9):
            wps = pp.tile([P, 512], f32, tag="ps")
            nc.tensor.matmul(wps[:], wsb[:, :P], wsb[:], start=True, stop=True)

        # DMA engines idle ~45% between transfers (turnaround), so weight
        # slabs ship fused: gate/up as groups [j0], [j1-2], [j3-4], [j5]
        # (the first solo so j0 can start early; the tail split so j3's
        # weights don't wait on j5's bytes), tokens as two contiguous
        # halves, one per HW queue.
        JG = [(0, 1), (1, 2), (3, 2), (5, 1)]  # (first j, group size)
        gts = [[None] * KF for _ in range(EPC)]  # (tile, idx-within-group)
        uts = [[None] * KF for _ in range(EPC)]
        dts = [[None] * 3 for _ in range(EPC)]
        xts = [[None] * 2, [None] * 2]  # 2 token halves per expert
        xns = (2, 2)

        def trig_gt(e, g, eng, w, store, wtag):
            j0, nj = JG[g]
            t = wgp.tile([P, nj, KD * P], bf16, tag=f"{wtag}g{g}", name=f"{wtag}g{g}")
            inst = eng.dma_start(t[:], w[e, j0 : j0 + nj].rearrange("j p c -> p j c"))
            for i in range(nj):
                store[e][j0 + i] = (t, i)
            return inst

        def trig_dt(e, h, eng):
            dt = wdp.tile([P, KF // 3, D], bf16, tag=f"dt{h}")
            inst = eng.dma_start(
                dt[:],
                wd[e].rearrange("(k p) d -> p k d", p=P)[:, bass.ts(h, KF // 3), :],
            )
            dts[e][h] = dt
            return inst

        def trig_xt(e, h, eng, ns):
            C = CS[e]
            xt = xp.tile([P, (KD // ns) * C], bf16, tag=f"xt{e}_{h}")
            inst = eng.dma_start(xt[:], xgts[e][:, bass.ts(h, (KD // ns) * C)])
            xts[e][h] = xt
            return inst

        # DMA engines pipeline packets only when the descriptor backlog is
        # deep, so everything is queued as early as possible; one token
        # half leads each HW queue so the first j-pair's data arrives
        # first. down(e0,h0) rides the SWDGE queue.
        trig_xt(0, 0, nc.sync, 2)
        trig_xt(0, 1, nc.scalar, 2)
        for g in range(len(JG)):
            trig_gt(0, g, nc.sync, wg, gts, "wg")
            trig_gt(0, g, nc.scalar, wu, uts, "wu")
        trig_dt(0, 0, nc.gpsimd)
        trig_dt(0, 1, nc.sync)
        for g in range(len(JG)):
            trig_gt(1, g, nc.sync, wg, gts, "wg")

        # triggers the scalar engine issues one-at-a-time between its
        # eviction copies during the e0 down phase (deadline order)
        e0down_trigs = [("xt", 0), ("xt", 1)] + [
            ("ut", g) for g in range(len(JG))
        ]

        def pop_trig():
            if e0down_trigs:
                kind, a = e0down_trigs.pop(0)
                if kind == "xt":
                    trig_xt(1, a, nc.scalar, 2)
                else:
                    trig_gt(1, a, nc.scalar, wu, uts, "wu")

        for e in range(EPC):
            C = CS[e]
            # ---- gate/up + SwiGLU -> hT [F, C] bf16 ----
            ht = hp.tile([P, KF, C], bf16, tag=f"ht{e}")
            for j in range(KF):
                gt, gi = gts[e][j]
                ut, ui = uts[e][j]
                g_ps = pp.tile([P, C], f32, tag="ps")
                u_ps = pp.tile([P, C], f32, tag="ps")
                kdn = KD // xns[e]
                for k in range(KD):
                    nc.tensor.matmul(
                        g_ps[:],
                        gt[:, gi, bass.ts(k, P)],
                        xts[e][k // kdn][:, bass.ts(k % kdn, C)],
                        start=(k == 0),
                        stop=(k == KD - 1),
                    )
                for k in range(KD):
                    nc.tensor.matmul(
                        u_ps[:],
                        ut[:, ui, bass.ts(k, P)],
                        xts[e][k // kdn][:, bass.ts(k % kdn, C)],
                        start=(k == 0),
                        stop=(k == KD - 1),
                    )
                sil = sp.tile([P, C], f32, tag="sil")
                nc.scalar.activation(
                    sil[:], g_ps[:], mybir.ActivationFunctionType.Silu
                )
                nc.vector.tensor_mul(ht[:, j, :], sil[:], u_ps[:])
            if e == 0:
                trig_dt(0, 2, nc.scalar)

            # ---- down proj: yT[m] = sum_h sum_{j in slab h} ----
            ydst = ys[e].rearrange("(m p) c -> p m c", p=P)
            ysb = op.tile([P, MD, C], bf16, tag=f"ysb{e}")
            if e == 0:
                # staged: 3 passes of 2 f-chunks, partials accumulated in
                # SBUF bf16 -- defers down-weight slab h's deadline to pass h,
                # which lets the input stream fit during the e0 phases.
                for h in range(3):
                    for m in range(MD):
                        y_ps = pp.tile([P, C], f32, tag="ps")
                        for i in range(2):
                            nc.tensor.matmul(
                                y_ps[:],
                                dts[e][h][:, i, bass.ts(m, P)],
                                ht[:, 2 * h + i, :],
                                start=(i == 0),
                                stop=(i == 1),
                            )
                        if h == 0:
                            # ACT engine owns the first partial (psum port)
                            nc.scalar.copy(ysb[:, m, :], y_ps[:])
                            if m % 2 == 1:
                                pop_trig()
                        else:
                            nc.vector.tensor_add(
                                ysb[:, m, :], ysb[:, m, :], y_ps[:]
                            )
                        if h == 2 and m % 4 == 3:
                            nc.gpsimd.dma_start(
                                ydst[:, m - 3 : m + 1, :], ysb[:, m - 3 : m + 1, :]
                            )
                    if h == 0:
                        while e0down_trigs:
                            pop_trig()
                        # e1 down slabs join the queues here; WAR-gated on
                        # e0's pass reads (wdp bufs=1) so they stream during
                        # e1's gate phase, not before. Slab 0 rides the
                        # otherwise-idle SWDGE queue.
                        trig_dt(1, 0, nc.gpsimd)
                        trig_dt(1, 1, nc.sync)
                        trig_dt(1, 2, nc.scalar)
            else:
                # input is all on-chip by now: two subrounds of 8 m-chunks,
                # full 6-tile accumulation in PSUM (one eviction per m-chunk,
                # split across DVE and ACT so neither gates the PE).
                psub = [None] * 8
                for m0 in range(0, MD, 8):
                    for h in range(3):
                        for m in range(m0, m0 + 8):
                            y_ps = psub[m - m0] if h else pp.tile(
                                [P, C], f32, tag="ps"
                            )
                            if h == 0:
                                psub[m - m0] = y_ps
                            for i in range(2):
                                nc.tensor.matmul(
                                    y_ps[:],
                                    dts[e][h][:, i, bass.ts(m, P)],
                                    ht[:, 2 * h + i, :],
                                    start=(h == 0 and i == 0),
                                    stop=(h == 2 and i == 1),
                                )
                            if h == 2:
                                ev = nc.vector.tensor_copy if m % 2 else nc.scalar.copy
                                ev(ysb[:, m, :], y_ps[:])
                            if h == 2 and m % 2 == 1:
                                yeng = nc.sync if (m // 2) % 2 == 0 else nc.scalar
                                yeng.dma_start(
                                    ydst[:, m - 1 : m + 1, :], ysb[:, m - 1 : m + 1, :]
                                )

    _split_waits(nc)
    return nc


_CACHE = {}


def _get_nc(C0, C1):
    if (C0, C1) not in _CACHE:
        _CACHE[(C0, C1)] = build_moe(C0, C1)
    return _CACHE[(C0, C1)]


def _route(x, router_w):
    """Replicates the reference router in f32: softmax over expert scores,
    top-2, renormalize."""
    xf = x.reshape(-1, D).astype(np.float32)
    scores = xf @ router_w.astype(np.float32)
    m = scores.max(axis=-1, keepdims=True)
    ex = np.exp(scores - m)
    probs = ex / ex.sum(axis=-1, keepdims=True)
    idx = np.argsort(-probs, axis=-1, kind="stable")[:, :TOPK]
    wts = np.take_along_axis(probs, idx, axis=-1)
    wts = wts / wts.sum(axis=-1, keepdims=True)
    return idx.astype(np.int32), wts.astype(np.float32)


def _cap(n):
    return min(512, max(P, -(-n // 8) * 8))


def kernel(x, router_w, gate_w, up_w, down_w):
    import ml_dtypes

    bf = ml_dtypes.bfloat16

    x = np.asarray(x)
    in_dtype = x.dtype
    xf = x.reshape(-1, D).astype(np.float32)
    idx, wts = _route(x, np.asarray(router_w))

    # token lists per expert
    tok_ids = [None] * E
    tok_wts = [None] * E
    counts = np.zeros(E, dtype=np.int64)
    for e in range(E):
        sel = np.nonzero(idx == e)
        tok_ids[e] = sel[0].astype(np.int64)
        tok_wts[e] = wts[sel[0], sel[1]]
        counts[e] = len(tok_ids[e])

    # heaviest 8 experts -> slot 0 (capacity C0), lightest 8 -> slot 1 (C1)
    order = np.argsort(-counts, kind="stable")
    slot_exp = [(int(order[c]), int(order[8 + c])) for c in range(NCORES)]
    C0 = _cap(int(counts[order[0]]))
    C1 = _cap(int(counts[order[8]]))

    nc = _get_nc(C0, C1)

    KD, KF = D // P, F // P

    def tile_gateup(w):
        # [E, D, F] -> [E, KF, P, KD*P] with w_t[e,j,p,k*P+f] = w[e,k*P+p,j*P+f]
        w = np.asarray(w).astype(bf)
        w = w.reshape(E, KD, P, KF, P).transpose(0, 3, 2, 1, 4)
        return np.ascontiguousarray(w.reshape(E, KF, P, KD * P))

    g16 = tile_gateup(gate_w)
    u16 = tile_gateup(up_w)
    d16 = np.asarray(down_w).astype(bf)
    xT = np.ascontiguousarray(xf.T)  # [D, B*T] f32

    in_maps = []
    for c in range(NCORES):
        im = {}
        eids = slot_exp[c]
        for s, C in ((0, C0), (1, C1)):
            e = eids[s]
            n = int(counts[e])
            xg = np.zeros((P, KD, C), dtype=bf)
            gath = xT[:, tok_ids[e]]  # [D, n] f32
            xg[:, :, :n] = gath.astype(bf).reshape(KD, P, n).transpose(1, 0, 2)
            im[f"xgt{s}"] = xg.reshape(P, KD * C)
        im["wg"] = np.ascontiguousarray(g16[list(eids)])
        im["wu"] = np.ascontiguousarray(u16[list(eids)])
        im["wd"] = np.ascontiguousarray(d16[list(eids)])
        in_maps.append(im)

    res = run_bass_kernel_spmd(nc, in_maps, list(range(NCORES)))

    out = np.zeros((B * T, D), dtype=np.float32)
    for c in range(NCORES):
        for s in range(EPC):
            e = slot_exp[c][s]
            n = int(counts[e])
            yv = res.results[c][f"y{s}"]  # [D, C] bf16
            out[tok_ids[e]] += tok_wts[e][:, None] * yv[:, :n].astype(np.float32).T
    return out.reshape(B, T, D).astype(in_dtype)

